# revision 44
# baseline (speedup 1.0000x reference)
"""Falcon-style MQA attention (71 heads, 1 KV head, RoPE, causal) on 8 TRN2 NeuronCores.

Sharding: tensor-parallel over query heads (9 slots per core, core 7 has 8 real
+ 1 zero-pad), single KV head replicated. Per core: QKV projection for its
heads (+KV), RoPE, causal attention in transposed [k, q] layout, then a PARTIAL
dense projection over the core's own head rows for all 4544 output columns.
The host sums the 8 bf16 partial outputs (contraction-sharded dense).

v2: software-pipelined to keep the tensor engine continuously busy (HAM warm):
 - QKV+RoPE pipelined per 256-token chunk
 - remaining QKV / dense GEMMs interleaved as PE filler inside attention
 - reciprocal_approx_fast for softmax denominators (was 3.3us/call reciprocal)
 - bf16 moving operands on all hot matmuls; elementwise spread over Act/DVE/GpSimd
 - coarse 3D DMAs; bf16 output partials summed on host
 - wq/hs SBUF region reused for wd (staged pools)

Self-contained: hardcodes all shapes; needs only numpy + ml_dtypes + concourse.
"""

import math
from contextlib import ExitStack

import numpy as np
import ml_dtypes

import concourse.bass as bass
import concourse.mybir as mybir
import concourse.tile as tile
from concourse import bacc
from concourse.bass_utils import run_bass_kernel_spmd

NCORES = 8
N, L, D = 2, 1024, 4544
H, DKV = 71, 64
M = N * L                    # 2048 tokens
DP = 4608                    # D padded to 36*128
KT = DP // 128               # 36 contraction tiles for QKV
HPC = 9                      # head slots per core (core 7: 8 real + 1 zero-pad)
QROWS = HPC * DKV            # 576 q rows per core
QPAD = 640                   # padded to 5*128 for the dense contraction
RROWS = QROWS + 2 * DKV      # 704 fused rows per core (q + k + v)
RC = 6                       # row-chunks of the QKV output (5x128 + 64)
MCH = 512                    # QKV token-chunk width (4 chunks)
NMC = M // MCH
CPB = NMC // N               # chunks per batch
JPC = MCH // 128             # j-tiles per chunk
ROPE_BASE = 10000.0

F32 = mybir.dt.float32
F32R = mybir.dt.float32r
BF16 = mybir.dt.bfloat16
COPYF = mybir.ActivationFunctionType.Copy
EXPF = mybir.ActivationFunctionType.Exp

_DEBUG_DUMPS = False


def _build():
    nc = bacc.Bacc("TRN2", target_bir_lowering=False, debug=False, num_devices=NCORES)

    hs_bf = nc.dram_tensor("hs_bf", [DP, M], BF16, kind="ExternalInput")      # hs.T
    wq_bf = nc.dram_tensor("wq_bf", [DP, RROWS], BF16, kind="ExternalInput")  # wq_loc.T
    wd_bf = nc.dram_tensor("wd_bf", [QPAD, D], BF16, kind="ExternalInput")    # wd rows for local heads
    cos_in = nc.dram_tensor("cos_in", [128, L], F32, kind="ExternalInput")
    sin_in = nc.dram_tensor("sin_in", [128, L], F32, kind="ExternalInput")
    tri_in = nc.dram_tensor("tri", [128, 128], BF16, kind="ExternalInput")
    prope_in = nc.dram_tensor("prope_in", [128, 128], F32R, kind="ExternalInput")
    id64_in = nc.dram_tensor("id64_in", [64, 64], F32R, kind="ExternalInput")
    colones_in = nc.dram_tensor("colones_in", [128, 16], BF16, kind="ExternalInput")
    out = nc.dram_tensor("out", [M, D], BF16, kind="ExternalOutput")
    if _DEBUG_DUMPS:
        dbg_qk = nc.dram_tensor("dbg_qk", [128, 5 * M], BF16, kind="ExternalOutput")
        dbg_attn = nc.dram_tensor("dbg_attn", [128, 5 * M], BF16, kind="ExternalOutput")
        dbg_vnat = nc.dram_tensor("dbg_vnat", [128, N * 8 * (DKV + 1)], BF16,
                                  kind="ExternalOutput")
        dbg_kd = nc.dram_tensor("dbg_kd", [128, N * L], BF16, kind="ExternalOutput")

    with tile.TileContext(nc) as tc, ExitStack() as top:
        constp = top.enter_context(tc.tile_pool(name="const", bufs=1))
        workp = top.enter_context(tc.tile_pool(name="work", bufs=2))
        finp = top.enter_context(tc.tile_pool(name="fin", bufs=1))
        otp = top.enter_context(tc.tile_pool(name="otp", bufs=3))
        expp = top.enter_context(tc.tile_pool(name="exps", bufs=6))
        psG = top.enter_context(tc.tile_pool(name="psG", bufs=2, space="PSUM"))
        psS = top.enter_context(tc.tile_pool(name="psS", bufs=3, space="PSUM"))
        psV = top.enter_context(tc.tile_pool(name="psV", bufs=3, space="PSUM"))

        # ---- constants ----
        cosb = constp.tile([128, L], F32)
        sinb = constp.tile([128, L], F32)
        trib = constp.tile([128, 128], BF16)
        prope = constp.tile([128, 128], F32R)
        id64 = constp.tile([64, 64], F32R)
        nc.sync.dma_start(cosb[:], cos_in[:])
        nc.sync.dma_start(sinb[:], sin_in[:])
        nc.sync.dma_start(trib[:], tri_in[:])
        nc.sync.dma_start(prope[:], prope_in[:])
        nc.sync.dma_start(id64[:], id64_in[:])

        # ---- persistent SBUF state ----
        bigp = top.enter_context(tc.tile_pool(name="big", bufs=1))
        qk_bf = bigp.tile([128, 5, M], BF16)          # rope'd q (+k in chunk 4 hi)
        attn_sb = bigp.tile([128, 5, M], BF16)        # attention out rows
        v_nat = bigp.tile([128, N * 8, DKV + 1], BF16)  # v.T per j-tile + ones col
        kT_dup = bigp.tile([128, N, L], BF16)         # k duplicated on both halves

        nc.sync.dma_start(v_nat[:, :, DKV:DKV + 1],
                          colones_in[:].rearrange("p (s o) -> p s o", o=1))
        nc.vector.memset(attn_sb[64:128, 4, :], 0.0)

        # stage A: QKV weights + hs chunks (region later reused for wd)
        stageA = ExitStack()
        wqp = stageA.enter_context(tc.tile_pool(name="wqp", bufs=1))
        hstp = stageA.enter_context(tc.tile_pool(name="hst", bufs=2))
        wqT = wqp.tile([128, KT, RROWS], BF16)
        wq_r = wq_bf[:].rearrange("(kt p) r -> p kt r", p=128)
        for k0 in range(0, KT, 9):
            nc.sync.dma_start(wqT[:, k0:k0 + 9, :], wq_r[:, k0:k0 + 9, :])

        hs_r = hs_bf[:].rearrange("(kt p) m -> p kt m", p=128)

        # ---------- stage-1 unit generators (QKV + RoPE per chunk) ----------
        def qkv_chunk(mc):
            """Generator: QKV projection + RoPE + v prep for token chunk mc."""
            n, loc0 = mc // CPB, MCH * (mc % CPB)
            sl = slice(MCH * mc, MCH * (mc + 1))
            hsT = hstp.tile([128, KT, MCH], BF16, tag="hsT")
            for k0 in range(0, KT, 9):
                nc.sync.dma_start(hsT[:, k0:k0 + 9, :], hs_r[:, k0:k0 + 9, sl])
            for rc in range(RC):
                rp = 128 if rc < 5 else 64
                ps = psG.tile([128, 512], F32, tag="g")
                for kt in range(KT):
                    nc.tensor.matmul(
                        ps[:rp, :MCH], wqT[:, kt, 128 * rc:128 * rc + rp],
                        hsT[:, kt, :], start=(kt == 0), stop=(kt == KT - 1))
                    if kt % 12 == 11 and kt != KT - 1:
                        yield
                if rc < 5:
                    # rope: qk = x*cos + (P x)*sin  (256-wide halves)
                    for hh in range(MCH // 256):
                        hsl = slice(256 * hh, 256 * (hh + 1))
                        gsl = slice(MCH * mc + 256 * hh, MCH * mc + 256 * (hh + 1))
                        csl = slice(loc0 + 256 * hh, loc0 + 256 * (hh + 1))
                        xb = workp.tile([128, 256], F32R, tag="xb")
                        nc.scalar.activation(xb[:], ps[:, hsl], COPYF)
                        pp = psS.tile([128, 256], F32, tag="s")
                        nc.tensor.matmul(pp[:], prope[:], xb[:],
                                         start=True, stop=True)
                        a = workp.tile([128, 256], F32, tag="ra")
                        nc.gpsimd.tensor_mul(a[:], xb[:], cosb[:, csl])
                        b = workp.tile([128, 256], F32, tag="rb")
                        nc.vector.tensor_mul(b[:], pp[:], sinb[:, csl])
                        nc.vector.tensor_add(qk_bf[:, rc, gsl], a[:], b[:])
                else:
                    # v rows -> transposed into v_nat j-tiles
                    for hh in range(MCH // 256):
                        vsb = workp.tile([64, 256], F32R, tag="vsb")
                        nc.scalar.activation(
                            vsb[:], ps[0:64, 256 * hh:256 * (hh + 1)], COPYF)
                        for half in range(2):
                            tp = psS.tile([128, 64], F32R, tag="s")
                            nc.tensor.transpose(
                                tp[:], vsb[:, 128 * half:128 * (half + 1)], id64[:])
                            nc.scalar.activation(
                                v_nat[:, JPC * mc + 2 * hh + half, 0:DKV],
                                tp[:], COPYF)
                yield
            if mc % CPB == CPB - 1:
                # k of batch n complete: build kT_dup (both halves)
                nsl = slice(L * n, L * (n + 1))
                nc.scalar.dma_start(kT_dup[0:64, n, :], qk_bf[64:128, 4, nsl])
                nc.scalar.dma_start(kT_dup[64:128, n, :], qk_bf[64:128, 4, nsl])
                yield

        # ---------- attention per head ----------
        def attn_head(n, h):
            """Generator: one attention head, fine-grained yields. Score (A)
            and AV (B) emissions yield separately so a lockstep pair driver
            lands both heads' 64-contraction score matmuls adjacent in the PE
            queue (row-groups 0-1 / 2-3 run concurrently)."""
            poff = (64 * h) % 128
            prc = (64 * h) // 128
            kTn = kT_dup[poff:poff + 64, n, :]
            qh = qk_bf[poff:poff + 64, prc, L * n:L * (n + 1)]
            for qc in range(2):
                av = psV.tile([65, 512], F32, tag="av")
                njt = 4 * (qc + 1)
                pend = None
                for jt in range(njt):
                    off = max(0, 128 * jt - 512 * qc)
                    sp = psS.tile([128, 512], F32, tag="s")
                    nc.tensor.matmul(
                        sp[:, 0:512 - off],
                        kTn[:, 128 * jt:128 * (jt + 1)],
                        qh[:, 512 * qc + off:512 * (qc + 1)],
                        start=True, stop=True)
                    et = expp.tile([128, 512], BF16, tag="exp")
                    nc.scalar.activation(
                        et[:, off:512], sp[:, 0:512 - off], EXPF,
                        scale=1.0 / math.sqrt(DKV))
                    if 128 * jt >= 512 * qc:
                        nc.gpsimd.tensor_mul(
                            et[:, off:off + 128], et[:, off:off + 128], trib[:])
                    yield
                    if pend is not None:
                        pjt, po, pet = pend
                        nc.tensor.matmul(
                            av[:, po:512], v_nat[:, 8 * n + pjt, :], pet[:, po:512],
                            start=(pjt == 0), stop=False)
                    pend = (jt, off, et)
                    yield
                pjt, po, pet = pend
                nc.tensor.matmul(
                    av[:, po:512], v_nat[:, 8 * n + pjt, :], pet[:, po:512],
                    start=(pjt == 0), stop=True)
                den = finp.tile([1, 512], F32, tag="den")
                nc.vector.tensor_copy(den[:], av[64:65, :])
                rec32 = finp.tile([1, 512], F32, tag="rec32")
                with nc.allow_low_precision(reason="softmax denom"):
                    nc.vector.reciprocal_approx_fast(rec32[:], den[:])
                yield
                prb = finp.tile([64, 512], F32, tag="prb")
                nc.gpsimd.partition_broadcast(prb[:], rec32[0:1, :])
                ob32 = finp.tile([64, 512], F32, tag="ob32")
                nc.scalar.activation(ob32[:], av[0:64, :], COPYF)
                osl = slice(L * n + 512 * qc, L * n + 512 * (qc + 1))
                nc.vector.tensor_mul(
                    attn_sb[poff:poff + 64, prc, osl], ob32[:], prb[:])
                if h == 8:
                    # replicate head-8 rows into the pad partitions for the
                    # paired dense chunk-4 matmuls
                    nc.vector.tensor_copy(
                        attn_sb[64:128, 4, osl], attn_sb[0:64, 4, osl])
                yield

        def attn_pair(n, h0, h1):
            """Lockstep pair driver: both heads advance one unit per round."""
            g0, g1 = attn_head(n, h0), attn_head(n, h1)
            while True:
                r0 = next(g0, "end")
                r1 = next(g1, "end")
                if r0 == "end" and r1 == "end":
                    return
                yield

        # ---------- dense units ----------
        CCH = [512] * 8 + [448]          # dense column chunks (sum = 4544)

        def dense_unit(wdT, n, mp, col, w):
            """Two m-tiles per unit; their half-contraction chunk-4 matmuls
            sit at row groups 0-1 / 2-3 and run concurrently."""
            mta, mtb = 2 * mp, 2 * mp + 1
            wa = slice(L * n + 128 * mta, L * n + 128 * (mta + 1))
            wb = slice(L * n + 128 * mtb, L * n + 128 * (mtb + 1))
            pa = psG.tile([128, 512], F32, tag="g")
            pb = psS.tile([128, 512], F32, tag="s")
            for kt in range(4):
                nc.tensor.matmul(pa[:, :w], attn_sb[:, kt, wa],
                                 wdT[:, kt, col:col + w],
                                 start=(kt == 0), stop=False)
            for kt in range(4):
                nc.tensor.matmul(pb[:, :w], attn_sb[:, kt, wb],
                                 wdT[:, kt, col:col + w],
                                 start=(kt == 0), stop=False)
            nc.tensor.matmul(pa[:, :w], attn_sb[0:64, 4, wa],
                             wdT[0:64, 4, col:col + w], start=False, stop=True)
            nc.tensor.matmul(pb[:, :w], attn_sb[64:128, 4, wb],
                             wdT[64:128, 4, col:col + w], start=False, stop=True)
            ota = otp.tile([128, 512], BF16, tag="ot")
            nc.scalar.activation(ota[:, :w], pa[:, :w], COPYF)
            nc.sync.dma_start(out[wa, col:col + w], ota[:, :w])
            otb = otp.tile([128, 512], BF16, tag="ot")
            nc.vector.tensor_copy(otb[:, :w], pb[:, :w])
            nc.sync.dma_start(out[wb, col:col + w], otb[:, :w])

        def dense_units(wdT, n):
            for mp in range(4):
                col = 0
                for w in CCH:
                    yield (lambda n=n, mp=mp, col=col, w=w:
                           dense_unit(wdT, n, mp, col, w))
                    col += w

        # ---------- drivers ----------
        def gen_queue_fillers(gens, n_units):
            """n_units filler thunks advancing the shared generator queue."""
            q = list(gens)
            def unit():
                while q:
                    try:
                        next(q[0])
                        return
                    except StopIteration:
                        q.pop(0)
            return [unit] * n_units

        def run_heads(n, fillers):
            """One head-pair in flight (2 av banks) with the solo 9th head
            riding alongside the first pair (3rd av bank); fillers uniform."""
            queue = [lambda h0=h0: attn_pair(n, h0, h0 + 1)
                     for h0 in range(0, HPC - 1, 2)]
            slots = [(queue.pop(0)(), True), (attn_head(n, HPC - 1), False)]
            est_rounds = 120
            pace = len(fillers) / est_rounds
            acc = 0.0
            fi = 0
            while slots:
                for item in list(slots):
                    try:
                        next(item[0])
                    except StopIteration:
                        slots.remove(item)
                        if queue and not any(p for _, p in slots):
                            slots.append((queue.pop(0)(), True))
                acc += pace
                while acc >= 1.0 and fi < len(fillers):
                    fillers[fi]()
                    fi += 1
                    acc -= 1.0
            while fi < len(fillers):
                fillers[fi]()
                fi += 1

        # stage 1: QKV+rope for batch 0, sequential
        for mc in range(CPB):
            for _ in qkv_chunk(mc):
                pass

        # batch-0 attention with batch-1 QKV as PE filler
        run_heads(0, gen_queue_fillers(
            [qkv_chunk(mc) for mc in range(CPB, NMC)], 37))

        # wq/hs dead; reuse their SBUF region for wd (WAR deps auto-inserted)
        stageA.close()
        stageB = ExitStack()
        wdp = stageB.enter_context(tc.tile_pool(name="wdp", bufs=1))
        wdT = wdp.tile([128, QPAD // 128, D], BF16)
        wd_r = wd_bf[:].rearrange("(kt p) c -> p kt c", p=128)
        wcol = 0
        for w in CCH:
            nc.sync.dma_start(wdT[:, :, wcol:wcol + w], wd_r[:, :, wcol:wcol + w])
            wcol += w

        # batch-1 attention with batch-0 dense as PE filler
        run_heads(1, list(dense_units(wdT, 0)))

        # drain: batch-1 dense
        for u in dense_units(wdT, 1):
            u()
        stageB.close()

        if _DEBUG_DUMPS:
            nc.sync.dma_start(
                dbg_qk[:].rearrange("p (c m) -> p c m", c=5), qk_bf[:])
            nc.sync.dma_start(
                dbg_attn[:].rearrange("p (c m) -> p c m", c=5), attn_sb[:])
            nc.sync.dma_start(
                dbg_vnat[:].rearrange("p (j d) -> p j d", j=N * 8), v_nat[:])
            nc.sync.dma_start(
                dbg_kd[:].rearrange("p (n l) -> p n l", n=N), kT_dup[:])

    nc.compile()
    return nc


_NC_CACHE = None


def _get_nc():
    global _NC_CACHE
    if _NC_CACHE is None:
        _NC_CACHE = _build()
    return _NC_CACHE


def _host_inputs(hidden_states, w_qkv, w_dense):
    """Build the per-core input maps (transpose + slice + bf16 cast on host)."""
    hs = np.asarray(hidden_states, dtype=np.float32).reshape(M, D)
    w_qkv = np.asarray(w_qkv, dtype=np.float32)
    w_dense = np.asarray(w_dense, dtype=np.float32)
    hs_bf = np.zeros((DP, M), dtype=ml_dtypes.bfloat16)
    hs_bf[:D, :] = np.ascontiguousarray(hs.T).astype(ml_dtypes.bfloat16)

    # RoPE tables, transposed to [dkv, l], duplicated on partitions 0-63 / 64-127
    inv_freq = 1.0 / (ROPE_BASE ** (np.arange(0, DKV, 2, dtype=np.float32) / DKV))
    t = np.arange(L, dtype=np.float32)
    freqs = np.outer(t, inv_freq)
    emb = np.concatenate([freqs, freqs], axis=-1)        # [L, DKV]
    cosT = np.cos(emb).T.astype(np.float32)              # [DKV, L]
    sinT = np.sin(emb).T.astype(np.float32)
    cos2 = np.concatenate([cosT, cosT], axis=0)
    sin2 = np.concatenate([sinT, sinT], axis=0)

    # tri[j, q] = 1 if j <= q (within-tile causal mask)
    tri = (np.arange(128)[:, None] <= np.arange(128)[None, :]).astype(
        ml_dtypes.bfloat16)

    # RoPE rotation: (P x)[d] = -x[d+32] (d<32), x[d-32] (d>=32); lhsT = P.T, 2 blocks
    P1 = np.zeros((DKV, DKV), dtype=np.float32)
    for d in range(32):
        P1[d, d + 32] = -1.0
        P1[d + 32, d] = 1.0
    PT = P1.T
    prope2 = np.zeros((128, 128), dtype=np.float32)
    prope2[:64, :64] = PT
    prope2[64:, 64:] = PT

    ident64 = np.eye(64, dtype=np.float32)

    kv_bf = w_qkv[H * DKV:, :].T.astype(ml_dtypes.bfloat16)   # [D, 128]
    in_maps = []
    for c in range(NCORES):
        h0 = HPC * c
        nh = min(HPC, H - h0)
        wq_loc = np.zeros((DP, RROWS), dtype=ml_dtypes.bfloat16)
        wq_loc[:D, :nh * DKV] = w_qkv[h0 * DKV:(h0 + nh) * DKV, :].T.astype(
            ml_dtypes.bfloat16)
        wq_loc[:D, QROWS:] = kv_bf

        # dense weight rows for this core's heads: w_dense columns
        # [64*h0 : 64*(h0+nh)) transposed, zero-padded to QPAD rows
        wd_loc = np.zeros((QPAD, D), dtype=ml_dtypes.bfloat16)
        wd_loc[:nh * DKV, :] = w_dense[:, DKV * h0:DKV * (h0 + nh)].T.astype(
            ml_dtypes.bfloat16)
        # replicate the 5th-chunk rows (head-slot 8) into the pad rows so the
        # paired dense chunk-4 matmul can read them at partitions 64-127
        wd_loc[QROWS:QPAD, :] = wd_loc[8 * DKV:QROWS, :]

        in_maps.append({
            "hs_bf": hs_bf,
            "wq_bf": wq_loc,
            "wd_bf": wd_loc,
            "cos_in": cos2,
            "sin_in": sin2,
            "tri": tri,
            "prope_in": prope2,
            "id64_in": ident64,
            "colones_in": np.ones((128, 16), dtype=ml_dtypes.bfloat16),
        })
    return in_maps


def kernel(hidden_states, w_qkv, w_dense, _trace=False, _trace_kwargs=None):
    nc = _get_nc()
    in_maps = _host_inputs(hidden_states, w_qkv, w_dense)
    kw = {}
    if _trace:
        kw = dict(trace=True, **(_trace_kwargs or {}))
    res = run_bass_kernel_spmd(nc, in_maps, list(range(NCORES)), **kw)
    full = res.results[0]["out"].astype(np.float32)
    for c in range(1, NCORES):
        full += res.results[c]["out"].astype(np.float32)
    kernel._last_exec_time_ns = res.exec_time_ns
    return full.reshape(N, L, D).astype(np.float32)


# revision 45
# speedup vs baseline: 1.5630x; 1.5630x over previous
"""Falcon-style MQA attention (71 heads, 1 KV head, RoPE, causal) on 8 TRN2 NeuronCores.

Sharding: tensor-parallel over query heads (9 slots per core, core 7 has 8 real
+ 1 zero-pad), single KV head replicated. Per core: QKV projection for its
heads (+KV), RoPE, causal attention in transposed [k, q] layout, then a PARTIAL
dense projection over the core's own head rows for all 4544 output columns.
The host sums the 8 bf16 partial outputs (contraction-sharded dense).

v2: software-pipelined to keep the tensor engine continuously busy (HAM warm):
 - QKV+RoPE pipelined per 256-token chunk
 - remaining QKV / dense GEMMs interleaved as PE filler inside attention
 - reciprocal_approx_fast for softmax denominators (was 3.3us/call reciprocal)
 - bf16 moving operands on all hot matmuls; elementwise spread over Act/DVE/GpSimd
 - coarse 3D DMAs; bf16 output partials summed on host
 - wq/hs SBUF region reused for wd (staged pools)

Self-contained: hardcodes all shapes; needs only numpy + ml_dtypes + concourse.
"""

import math
from contextlib import ExitStack

import numpy as np
import ml_dtypes

import concourse.bass as bass
import concourse.mybir as mybir
import concourse.tile as tile
from concourse import bacc
from concourse.bass_utils import run_bass_kernel_spmd

NCORES = 8
N, L, D = 2, 1024, 4544
H, DKV = 71, 64
M = N * L                    # 2048 tokens
DP = 4608                    # D padded to 36*128
KT = DP // 128               # 36 contraction tiles for QKV
HPC = 9                      # head slots per core (core 7: 8 real + 1 zero-pad)
QROWS = HPC * DKV            # 576 q rows per core
QPAD = 640                   # padded to 5*128 for the dense contraction
RROWS = QROWS + 2 * DKV      # 704 fused rows per core (q + k + v)
RC = 6                       # row-chunks of the QKV output (5x128 + 64)
MCH = 512                    # QKV token-chunk width (4 chunks)
NMC = M // MCH
CPB = NMC // N               # chunks per batch
JPC = MCH // 128             # j-tiles per chunk
ROPE_BASE = 10000.0

F32 = mybir.dt.float32
F32R = mybir.dt.float32r
BF16 = mybir.dt.bfloat16
COPYF = mybir.ActivationFunctionType.Copy
EXPF = mybir.ActivationFunctionType.Exp

_DEBUG_DUMPS = False


def _build():
    nc = bacc.Bacc("TRN2", target_bir_lowering=False, debug=False, num_devices=NCORES)

    hs_bf = nc.dram_tensor("hs_bf", [DP, M], BF16, kind="ExternalInput")      # hs.T
    wq_bf = nc.dram_tensor("wq_bf", [DP, RROWS], BF16, kind="ExternalInput")  # wq_loc.T
    wd_bf = nc.dram_tensor("wd_bf", [QPAD, D], BF16, kind="ExternalInput")    # wd rows for local heads
    cos_in = nc.dram_tensor("cos_in", [128, L], F32, kind="ExternalInput")
    sin_in = nc.dram_tensor("sin_in", [128, L], F32, kind="ExternalInput")
    tri_in = nc.dram_tensor("tri", [128, 128], BF16, kind="ExternalInput")
    prope_in = nc.dram_tensor("prope_in", [128, 128], F32R, kind="ExternalInput")
    id64_in = nc.dram_tensor("id64_in", [64, 64], F32R, kind="ExternalInput")
    colones_in = nc.dram_tensor("colones_in", [128, 16], BF16, kind="ExternalInput")
    out = nc.dram_tensor("out", [M, D], BF16, kind="ExternalOutput")
    if _DEBUG_DUMPS:
        dbg_qk = nc.dram_tensor("dbg_qk", [128, 5 * M], BF16, kind="ExternalOutput")
        dbg_attn = nc.dram_tensor("dbg_attn", [128, 5 * M], BF16, kind="ExternalOutput")
        dbg_vnat = nc.dram_tensor("dbg_vnat", [128, N * 8 * (DKV + 1)], BF16,
                                  kind="ExternalOutput")
        dbg_kd = nc.dram_tensor("dbg_kd", [128, N * L], BF16, kind="ExternalOutput")

    with tile.TileContext(nc) as tc, ExitStack() as top:
        constp = top.enter_context(tc.tile_pool(name="const", bufs=1))
        workp = top.enter_context(tc.tile_pool(name="work", bufs=2))
        finp = top.enter_context(tc.tile_pool(name="fin", bufs=1))
        otp = top.enter_context(tc.tile_pool(name="otp", bufs=3))
        expp = top.enter_context(tc.tile_pool(name="exps", bufs=6))
        psG = top.enter_context(tc.tile_pool(name="psG", bufs=2, space="PSUM"))
        psS = top.enter_context(tc.tile_pool(name="psS", bufs=3, space="PSUM"))
        psV = top.enter_context(tc.tile_pool(name="psV", bufs=3, space="PSUM"))

        # ---- constants ----
        cosb = constp.tile([128, L], F32)
        sinb = constp.tile([128, L], F32)
        trib = constp.tile([128, 128], BF16)
        prope = constp.tile([128, 128], F32R)
        id64 = constp.tile([64, 64], F32R)
        nc.sync.dma_start(cosb[:], cos_in[:])
        nc.sync.dma_start(sinb[:], sin_in[:])
        nc.sync.dma_start(trib[:], tri_in[:])
        nc.sync.dma_start(prope[:], prope_in[:])
        nc.sync.dma_start(id64[:], id64_in[:])

        # ---- persistent SBUF state ----
        bigp = top.enter_context(tc.tile_pool(name="big", bufs=1))
        qk_bf = bigp.tile([128, 5, M], BF16)          # rope'd q (+k in chunk 4 hi)
        attn_sb = bigp.tile([128, 5, M], BF16)        # attention out rows
        v_nat = bigp.tile([128, N * 8, DKV + 1], BF16)  # v.T per j-tile + ones col
        kT_dup = bigp.tile([128, N, L], BF16)         # k duplicated on both halves

        nc.sync.dma_start(v_nat[:, :, DKV:DKV + 1],
                          colones_in[:].rearrange("p (s o) -> p s o", o=1))
        nc.vector.memset(attn_sb[64:128, 4, :], 0.0)

        # stage A: QKV weights + hs chunks (region later reused for wd)
        stageA = ExitStack()
        wqp = stageA.enter_context(tc.tile_pool(name="wqp", bufs=1))
        hstp = stageA.enter_context(tc.tile_pool(name="hst", bufs=2))
        wqT = wqp.tile([128, KT, RROWS], BF16)
        wq_r = wq_bf[:].rearrange("(kt p) r -> p kt r", p=128)
        for k0 in range(0, KT, 9):
            nc.sync.dma_start(wqT[:, k0:k0 + 9, :], wq_r[:, k0:k0 + 9, :])

        hs_r = hs_bf[:].rearrange("(kt p) m -> p kt m", p=128)

        # ---------- stage-1 unit generators (QKV + RoPE per chunk) ----------
        def qkv_chunk(mc):
            """Generator: QKV projection + RoPE + v prep for token chunk mc."""
            n, loc0 = mc // CPB, MCH * (mc % CPB)
            sl = slice(MCH * mc, MCH * (mc + 1))
            hsT = hstp.tile([128, KT, MCH], BF16, tag="hsT")
            for k0 in range(0, KT, 9):
                nc.sync.dma_start(hsT[:, k0:k0 + 9, :], hs_r[:, k0:k0 + 9, sl])
            for rc in range(RC):
                rp = 128 if rc < 5 else 64
                ps = psG.tile([128, 512], F32, tag="g")
                for kt in range(KT):
                    nc.tensor.matmul(
                        ps[:rp, :MCH], wqT[:, kt, 128 * rc:128 * rc + rp],
                        hsT[:, kt, :], start=(kt == 0), stop=(kt == KT - 1))
                    if kt % 12 == 11 and kt != KT - 1:
                        yield
                if rc < 5:
                    # rope: qk = x*cos + (P x)*sin  (256-wide halves)
                    for hh in range(MCH // 256):
                        hsl = slice(256 * hh, 256 * (hh + 1))
                        gsl = slice(MCH * mc + 256 * hh, MCH * mc + 256 * (hh + 1))
                        csl = slice(loc0 + 256 * hh, loc0 + 256 * (hh + 1))
                        xb = workp.tile([128, 256], F32R, tag="xb")
                        nc.scalar.activation(xb[:], ps[:, hsl], COPYF)
                        pp = psS.tile([128, 256], F32, tag="s")
                        nc.tensor.matmul(pp[:], prope[:], xb[:],
                                         start=True, stop=True)
                        a = workp.tile([128, 256], F32, tag="ra")
                        nc.vector.tensor_mul(a[:], xb[:], cosb[:, csl])
                        b = workp.tile([128, 256], F32, tag="rb")
                        nc.vector.tensor_mul(b[:], pp[:], sinb[:, csl])
                        nc.vector.tensor_add(qk_bf[:, rc, gsl], a[:], b[:])
                else:
                    # v rows -> transposed into v_nat j-tiles
                    for hh in range(MCH // 256):
                        vsb = workp.tile([64, 256], F32R, tag="vsb")
                        nc.scalar.activation(
                            vsb[:], ps[0:64, 256 * hh:256 * (hh + 1)], COPYF)
                        for half in range(2):
                            tp = psS.tile([128, 64], F32R, tag="s")
                            nc.tensor.transpose(
                                tp[:], vsb[:, 128 * half:128 * (half + 1)], id64[:])
                            nc.scalar.activation(
                                v_nat[:, JPC * mc + 2 * hh + half, 0:DKV],
                                tp[:], COPYF)
                yield
            if mc % CPB == CPB - 1:
                # k of batch n complete: build kT_dup (both halves)
                nsl = slice(L * n, L * (n + 1))
                nc.scalar.dma_start(kT_dup[0:64, n, :], qk_bf[64:128, 4, nsl])
                nc.scalar.dma_start(kT_dup[64:128, n, :], qk_bf[64:128, 4, nsl])
                yield

        # ---------- attention per head ----------
        def attn_head(n, h):
            """Generator: one attention head, fine-grained yields. Score (A)
            and AV (B) emissions yield separately so a lockstep pair driver
            lands both heads' 64-contraction score matmuls adjacent in the PE
            queue (row-groups 0-1 / 2-3 run concurrently)."""
            poff = (64 * h) % 128
            prc = (64 * h) // 128
            kTn = kT_dup[poff:poff + 64, n, :]
            qh = qk_bf[poff:poff + 64, prc, L * n:L * (n + 1)]
            for qc in range(2):
                av = psV.tile([65, 512], F32, tag="av")
                njt = 4 * (qc + 1)
                pend = None
                for jt in range(njt):
                    off = max(0, 128 * jt - 512 * qc)
                    sp = psS.tile([128, 512], F32, tag="s")
                    nc.tensor.matmul(
                        sp[:, 0:512 - off],
                        kTn[:, 128 * jt:128 * (jt + 1)],
                        qh[:, 512 * qc + off:512 * (qc + 1)],
                        start=True, stop=True)
                    et = expp.tile([128, 512], BF16, tag="exp")
                    nc.scalar.activation(
                        et[:, off:512], sp[:, 0:512 - off], EXPF,
                        scale=1.0 / math.sqrt(DKV))
                    if 128 * jt >= 512 * qc:
                        nc.vector.tensor_mul(
                            et[:, off:off + 128], et[:, off:off + 128], trib[:])
                    yield
                    if pend is not None:
                        pjt, po, pet = pend
                        nc.tensor.matmul(
                            av[:, po:512], v_nat[:, 8 * n + pjt, :], pet[:, po:512],
                            start=(pjt == 0), stop=False)
                    pend = (jt, off, et)
                    yield
                pjt, po, pet = pend
                nc.tensor.matmul(
                    av[:, po:512], v_nat[:, 8 * n + pjt, :], pet[:, po:512],
                    start=(pjt == 0), stop=True)
                den = finp.tile([1, 512], F32, tag="den")
                nc.vector.tensor_copy(den[:], av[64:65, :])
                rec32 = finp.tile([1, 512], F32, tag="rec32")
                with nc.allow_low_precision(reason="softmax denom"):
                    nc.vector.reciprocal_approx_fast(rec32[:], den[:])
                yield
                prb = finp.tile([64, 512], F32, tag="prb")
                nc.gpsimd.partition_broadcast(prb[:], rec32[0:1, :])
                ob32 = finp.tile([64, 512], F32, tag="ob32")
                nc.scalar.activation(ob32[:], av[0:64, :], COPYF)
                osl = slice(L * n + 512 * qc, L * n + 512 * (qc + 1))
                nc.vector.tensor_mul(
                    attn_sb[poff:poff + 64, prc, osl], ob32[:], prb[:])
                if h == 8:
                    # replicate head-8 rows into the pad partitions for the
                    # paired dense chunk-4 matmuls
                    nc.vector.tensor_copy(
                        attn_sb[64:128, 4, osl], attn_sb[0:64, 4, osl])
                yield

        def attn_pair(n, h0, h1):
            """Lockstep pair driver: both heads advance one unit per round."""
            g0, g1 = attn_head(n, h0), attn_head(n, h1)
            while True:
                r0 = next(g0, "end")
                r1 = next(g1, "end")
                if r0 == "end" and r1 == "end":
                    return
                yield

        # ---------- dense units ----------
        CCH = [512] * 8 + [448]          # dense column chunks (sum = 4544)

        def dense_unit(wdT, n, mp, col, w):
            """Two m-tiles per unit; their half-contraction chunk-4 matmuls
            sit at row groups 0-1 / 2-3 and run concurrently."""
            mta, mtb = 2 * mp, 2 * mp + 1
            wa = slice(L * n + 128 * mta, L * n + 128 * (mta + 1))
            wb = slice(L * n + 128 * mtb, L * n + 128 * (mtb + 1))
            pa = psG.tile([128, 512], F32, tag="g")
            pb = psS.tile([128, 512], F32, tag="s")
            for kt in range(4):
                nc.tensor.matmul(pa[:, :w], attn_sb[:, kt, wa],
                                 wdT[:, kt, col:col + w],
                                 start=(kt == 0), stop=False)
            for kt in range(4):
                nc.tensor.matmul(pb[:, :w], attn_sb[:, kt, wb],
                                 wdT[:, kt, col:col + w],
                                 start=(kt == 0), stop=False)
            nc.tensor.matmul(pa[:, :w], attn_sb[0:64, 4, wa],
                             wdT[0:64, 4, col:col + w], start=False, stop=True)
            nc.tensor.matmul(pb[:, :w], attn_sb[64:128, 4, wb],
                             wdT[64:128, 4, col:col + w], start=False, stop=True)
            ota = otp.tile([128, 512], BF16, tag="ot")
            nc.scalar.activation(ota[:, :w], pa[:, :w], COPYF)
            nc.sync.dma_start(out[wa, col:col + w], ota[:, :w])
            otb = otp.tile([128, 512], BF16, tag="ot")
            nc.vector.tensor_copy(otb[:, :w], pb[:, :w])
            nc.sync.dma_start(out[wb, col:col + w], otb[:, :w])

        def dense_units(wdT, n):
            for mp in range(4):
                col = 0
                for w in CCH:
                    yield (lambda n=n, mp=mp, col=col, w=w:
                           dense_unit(wdT, n, mp, col, w))
                    col += w

        # ---------- drivers ----------
        def gen_queue_fillers(gens, n_units):
            """n_units filler thunks advancing the shared generator queue."""
            q = list(gens)
            def unit():
                while q:
                    try:
                        next(q[0])
                        return
                    except StopIteration:
                        q.pop(0)
            return [unit] * n_units

        def run_heads(n, fillers):
            """One head-pair in flight (2 av banks) with the solo 9th head
            riding alongside the first pair (3rd av bank); fillers uniform."""
            queue = [lambda h0=h0: attn_pair(n, h0, h0 + 1)
                     for h0 in range(0, HPC - 1, 2)]
            slots = [(queue.pop(0)(), True), (attn_head(n, HPC - 1), False)]
            est_rounds = 120
            pace = len(fillers) / est_rounds
            acc = 0.0
            fi = 0
            while slots:
                for item in list(slots):
                    try:
                        next(item[0])
                    except StopIteration:
                        slots.remove(item)
                        if queue and not any(p for _, p in slots):
                            slots.append((queue.pop(0)(), True))
                acc += pace
                while acc >= 1.0 and fi < len(fillers):
                    fillers[fi]()
                    fi += 1
                    acc -= 1.0
            while fi < len(fillers):
                fillers[fi]()
                fi += 1

        # stage 1: QKV+rope for batch 0, sequential
        for mc in range(CPB):
            for _ in qkv_chunk(mc):
                pass

        # batch-0 attention with batch-1 QKV as PE filler
        run_heads(0, gen_queue_fillers(
            [qkv_chunk(mc) for mc in range(CPB, NMC)], 37))

        # wq/hs dead; reuse their SBUF region for wd (WAR deps auto-inserted)
        stageA.close()
        stageB = ExitStack()
        wdp = stageB.enter_context(tc.tile_pool(name="wdp", bufs=1))
        wdT = wdp.tile([128, QPAD // 128, D], BF16)
        wd_r = wd_bf[:].rearrange("(kt p) c -> p kt c", p=128)
        wcol = 0
        for w in CCH:
            nc.sync.dma_start(wdT[:, :, wcol:wcol + w], wd_r[:, :, wcol:wcol + w])
            wcol += w

        # batch-1 attention with batch-0 dense as PE filler
        run_heads(1, list(dense_units(wdT, 0)))

        # drain: batch-1 dense
        for u in dense_units(wdT, 1):
            u()
        stageB.close()

        if _DEBUG_DUMPS:
            nc.sync.dma_start(
                dbg_qk[:].rearrange("p (c m) -> p c m", c=5), qk_bf[:])
            nc.sync.dma_start(
                dbg_attn[:].rearrange("p (c m) -> p c m", c=5), attn_sb[:])
            nc.sync.dma_start(
                dbg_vnat[:].rearrange("p (j d) -> p j d", j=N * 8), v_nat[:])
            nc.sync.dma_start(
                dbg_kd[:].rearrange("p (n l) -> p n l", n=N), kT_dup[:])

    nc.compile()
    return nc


_NC_CACHE = None


def _get_nc():
    global _NC_CACHE
    if _NC_CACHE is None:
        _NC_CACHE = _build()
    return _NC_CACHE


def _host_inputs(hidden_states, w_qkv, w_dense):
    """Build the per-core input maps (transpose + slice + bf16 cast on host)."""
    hs = np.asarray(hidden_states, dtype=np.float32).reshape(M, D)
    w_qkv = np.asarray(w_qkv, dtype=np.float32)
    w_dense = np.asarray(w_dense, dtype=np.float32)
    hs_bf = np.zeros((DP, M), dtype=ml_dtypes.bfloat16)
    hs_bf[:D, :] = np.ascontiguousarray(hs.T).astype(ml_dtypes.bfloat16)

    # RoPE tables, transposed to [dkv, l], duplicated on partitions 0-63 / 64-127
    inv_freq = 1.0 / (ROPE_BASE ** (np.arange(0, DKV, 2, dtype=np.float32) / DKV))
    t = np.arange(L, dtype=np.float32)
    freqs = np.outer(t, inv_freq)
    emb = np.concatenate([freqs, freqs], axis=-1)        # [L, DKV]
    cosT = np.cos(emb).T.astype(np.float32)              # [DKV, L]
    sinT = np.sin(emb).T.astype(np.float32)
    cos2 = np.concatenate([cosT, cosT], axis=0)
    sin2 = np.concatenate([sinT, sinT], axis=0)

    # tri[j, q] = 1 if j <= q (within-tile causal mask)
    tri = (np.arange(128)[:, None] <= np.arange(128)[None, :]).astype(
        ml_dtypes.bfloat16)

    # RoPE rotation: (P x)[d] = -x[d+32] (d<32), x[d-32] (d>=32); lhsT = P.T, 2 blocks
    P1 = np.zeros((DKV, DKV), dtype=np.float32)
    for d in range(32):
        P1[d, d + 32] = -1.0
        P1[d + 32, d] = 1.0
    PT = P1.T
    prope2 = np.zeros((128, 128), dtype=np.float32)
    prope2[:64, :64] = PT
    prope2[64:, 64:] = PT

    ident64 = np.eye(64, dtype=np.float32)

    kv_bf = w_qkv[H * DKV:, :].T.astype(ml_dtypes.bfloat16)   # [D, 128]
    in_maps = []
    for c in range(NCORES):
        h0 = HPC * c
        nh = min(HPC, H - h0)
        wq_loc = np.zeros((DP, RROWS), dtype=ml_dtypes.bfloat16)
        wq_loc[:D, :nh * DKV] = w_qkv[h0 * DKV:(h0 + nh) * DKV, :].T.astype(
            ml_dtypes.bfloat16)
        wq_loc[:D, QROWS:] = kv_bf

        # dense weight rows for this core's heads: w_dense columns
        # [64*h0 : 64*(h0+nh)) transposed, zero-padded to QPAD rows
        wd_loc = np.zeros((QPAD, D), dtype=ml_dtypes.bfloat16)
        wd_loc[:nh * DKV, :] = w_dense[:, DKV * h0:DKV * (h0 + nh)].T.astype(
            ml_dtypes.bfloat16)
        # replicate the 5th-chunk rows (head-slot 8) into the pad rows so the
        # paired dense chunk-4 matmul can read them at partitions 64-127
        wd_loc[QROWS:QPAD, :] = wd_loc[8 * DKV:QROWS, :]

        in_maps.append({
            "hs_bf": hs_bf,
            "wq_bf": wq_loc,
            "wd_bf": wd_loc,
            "cos_in": cos2,
            "sin_in": sin2,
            "tri": tri,
            "prope_in": prope2,
            "id64_in": ident64,
            "colones_in": np.ones((128, 16), dtype=ml_dtypes.bfloat16),
        })
    return in_maps


def kernel(hidden_states, w_qkv, w_dense, _trace=False, _trace_kwargs=None):
    nc = _get_nc()
    in_maps = _host_inputs(hidden_states, w_qkv, w_dense)
    kw = {}
    if _trace:
        kw = dict(trace=True, **(_trace_kwargs or {}))
    res = run_bass_kernel_spmd(nc, in_maps, list(range(NCORES)), **kw)
    full = res.results[0]["out"].astype(np.float32)
    for c in range(1, NCORES):
        full += res.results[c]["out"].astype(np.float32)
    kernel._last_exec_time_ns = res.exec_time_ns
    return full.reshape(N, L, D).astype(np.float32)
